# revision 1
# baseline (speedup 1.0000x reference)
"""Trainium2 Bass kernel for nn_ConvKernelBank.

Computation (see module docstring at bottom for reference semantics):
  alpha = softmax_M( causal_conv1d( gelu(pre_w @ mean_hw(q) + pre_b), mix_w ) + mix_b )
  k_out = sum_m alpha[b,m,t] * dwconv3d_causal(k, Wk[m])
  v_out = sum_m alpha[b,m,t] * dwconv3d_causal(v, Wv[m])

Strategy:
  - 8 NeuronCores, data-parallel over (batch, T-half): core i handles
    b = i // 2, t in [16*(i%2), 16*(i%2)+16).  Causal temporal halo of 2
    frames is passed in from the host (zeros at sequence start).
  - Per core everything lives in the natural layout [C=128 partitions,
    (t, h, w) free] which exactly fills the 128 SBUF partitions.
  - The mixture weights alpha[m, t] are folded into per-output-frame
    effective 27-tap depthwise filters W_eff[c, tap] (this turns
    3 convs + a mix into a single conv: 3x less work).
  - Each output frame is computed with 27 DVE scalar_tensor_tensor
    multiply-accumulate ops (per-partition scalar = per-channel filter
    tap), with boundary handling done by clipping the access patterns.
"""

import os
from contextlib import ExitStack

import numpy as np

import concourse.bass as bass
import concourse.tile as tile
from concourse import mybir
from concourse.bass_utils import run_bass_kernel_spmd
from concourse.vector_clock import ScopedClock

B, C, T, H, W = 4, 128, 32, 24, 24
M, KT, KS, MIXK = 3, 3, 3, 3
HW = H * W
NCORES = 8
TLOC = 16          # output frames per core
THALO = 2          # causal temporal halo
TIN = TLOC + THALO
F32 = mybir.dt.float32
NTAP = KT * KS * KS  # 27
# per-frame effective filter block: 27 k-taps then 27 v-taps
WBLK = 2 * NTAP      # 54

AluOp = mybir.AluOpType
ActFn = mybir.ActivationFunctionType


class _SplitDrainTileContext(tile.TileContext):
    """TileContext whose final drain splits semaphore waits across several
    drain instructions: this walrus build rejects >2 sync waits on one
    CTRL instruction ("Too many sync wait commands")."""

    MAX_WAITS = 1

    def _drain_and_barrier(self, tick_clock, wait_clock):
        nc = self.nc
        drain_inst = nc.sync.drain()
        wait_clock.add_sem_waits(
            drain_inst.ins, ScopedClock({None: tick_clock.global_clock})
        )
        mi = drain_inst.ins
        si = mi.sync_info
        waits = list(si.on_wait or []) if si is not None else []
        if len(waits) > self.MAX_WAITS:
            si.on_wait = waits[: self.MAX_WAITS]
            rest = waits[self.MAX_WAITS :]
            for i in range(0, len(rest), self.MAX_WAITS):
                d2 = nc.sync.drain()
                d2.ins.sync_info = mybir.SyncInfo(
                    on_wait=rest[i : i + self.MAX_WAITS], on_update=[]
                )
        nc.all_engine_barrier()
        popped = nc._tile_sem_poison_stack.pop()
        assert popped is self._sem_poison
        nc.clear_and_free_semaphores(list(self.sems.allocated().values()))
        nc.all_engine_barrier()


_MAX_SYNC_WAITS = 1

_NOP_ENGINES = {
    mybir.EngineType.PE,
    mybir.EngineType.DVE,
    mybir.EngineType.Activation,
    mybir.EngineType.Pool,
    mybir.EngineType.SP,
}


def _split_sync_waits(nc: bass.Bass, max_waits: int = _MAX_SYNC_WAITS) -> None:
    """Walrus rejects instructions carrying more than ~2 semaphore waits.
    Move excess waits onto freshly inserted same-engine NoOps placed just
    before the offending instruction (waiting earlier is always safe)."""
    for fn in nc.m.functions:
        for bb in fn.blocks:
            new_list = []
            changed = False
            for inst in bb.instructions:
                si = inst.sync_info
                waits = list(si.on_wait) if (si is not None and si.on_wait) else []
                if len(waits) > max_waits:
                    assert inst.engine in _NOP_ENGINES, (
                        f"can't split waits on {inst.engine} {type(inst).__name__}"
                    )
                    excess, keep = waits[:-max_waits], waits[-max_waits:]
                    for i in range(0, len(excess), max_waits):
                        nop = mybir.InstNoOp(
                            name=nc.get_next_instruction_name(), ins=[], outs=[]
                        )
                        nop.engine = inst.engine
                        nop.sync_info = mybir.SyncInfo(
                            on_wait=excess[i : i + max_waits], on_update=[]
                        )
                        new_list.append(nop)
                    si.on_wait = keep
                    changed = True
                new_list.append(inst)
            if changed:
                bb.instructions[:] = new_list


def _build_program() -> bass.Bass:
    nc = bass.Bass()

    qin = nc.declare_dram_parameter("qin", [C, TIN * HW], F32, isOutput=False)
    kin = nc.declare_dram_parameter("kin", [C, TIN * HW], F32, isOutput=False)
    vin = nc.declare_dram_parameter("vin", [C, TIN * HW], F32, isOutput=False)
    # [c, m*54 + tap] : tap 0..26 = Wk[m,c], 27..53 = Wv[m,c]
    wkv = nc.declare_dram_parameter("wkv", [C, M * WBLK], F32, isOutput=False)
    # (pre_w / HW).T  -> lhsT layout [c_in, c_out]
    prew = nc.declare_dram_parameter("prew", [C, C], F32, isOutput=False)
    preb = nc.declare_dram_parameter("preb", [C, 1], F32, isOutput=False)
    # [c, j*3 + m] = mix_w[m, c, j]
    mixw = nc.declare_dram_parameter("mixw", [C, MIXK * M], F32, isOutput=False)
    # mix_b tiled to [TLOC, M]
    mixb = nc.declare_dram_parameter("mixb", [TLOC, M], F32, isOutput=False)
    # halo validity mask for the mix-conv input h (0 for the 2 halo columns
    # on sequence-start cores, 1 elsewhere): [1, THALO] broadcast on host
    hmask = nc.declare_dram_parameter("hmask", [C, THALO], F32, isOutput=False)

    kout = nc.declare_dram_parameter("kout", [C, TLOC * HW], F32, isOutput=True)
    vout = nc.declare_dram_parameter("vout", [C, TLOC * HW], F32, isOutput=True)

    # scratch DRAM for the alpha partition-broadcast round trip
    adram = nc.dram_tensor("alpha_rt", [TLOC * M], F32)

    with ExitStack() as ctx:
        tc = ctx.enter_context(_SplitDrainTileContext(nc))

        consts = ctx.enter_context(tc.tile_pool(name="consts", bufs=1))
        big = ctx.enter_context(tc.tile_pool(name="big", bufs=1))
        small = ctx.enter_context(tc.tile_pool(name="small", bufs=1))
        outp = ctx.enter_context(tc.tile_pool(name="outp", bufs=4))
        psum = ctx.enter_context(tc.tile_pool(name="psum", bufs=2, space="PSUM"))

        # ---- load constants ----
        wkv_sb = consts.tile([C, M * WBLK], F32)
        nc.sync.dma_start(wkv_sb[:], wkv[:])
        prew_sb = consts.tile([C, C], F32)
        nc.sync.dma_start(prew_sb[:], prew[:])
        preb_sb = consts.tile([C, 1], F32)
        nc.sync.dma_start(preb_sb[:], preb[:])
        mixw_sb = consts.tile([C, MIXK * M], F32)
        nc.sync.dma_start(mixw_sb[:], mixw[:])
        mixb_sb = consts.tile([TLOC, M], F32)
        nc.sync.dma_start(mixb_sb[:], mixb[:])
        hmask_sb = consts.tile([C, THALO], F32)
        nc.sync.dma_start(hmask_sb[:], hmask[:])

        # ---- load bulk data ----
        q_sb = big.tile([C, TIN * HW], F32)
        nc.sync.dma_start(q_sb[:], qin[:])
        k_sb = big.tile([C, TIN * HW], F32)
        nc.sync.dma_start(k_sb[:], kin[:])
        v_sb = big.tile([C, TIN * HW], F32)
        nc.sync.dma_start(v_sb[:], vin[:])

        # ---- mix predictor ----
        # qg[c, t] = sum_hw q / HW   (the 1/HW is folded into prew on host).
        # Run the pooling on the (otherwise idle) ACT engine via accum_out
        # so the DVE stays free for the conv MACs.
        qg = small.tile([C, TIN], F32)
        qscratch = small.tile([C, HW], F32)
        for t in range(TIN):
            nc.scalar.activation(
                qscratch[:],
                q_sb[:, t * HW : (t + 1) * HW],
                ActFn.Copy,
                accum_out=qg[:, t : t + 1],
            )
        # h = gelu(prew.T @ qg + preb)
        h_ps = psum.tile([C, TIN], F32)
        nc.tensor.matmul(h_ps[:], prew_sb[:], qg[:], start=True, stop=True)
        h_sb = small.tile([C, TIN], F32)
        nc.scalar.activation(
            h_sb[:], h_ps[:], ActFn.Gelu, bias=preb_sb[:, 0:1], scale=1.0
        )
        # zero the causal halo columns where the reference zero-pads h
        nc.vector.tensor_mul(h_sb[:, 0:THALO], h_sb[:, 0:THALO], hmask_sb[:])

        # logits[t, m] = sum_j sum_c mix_w[m,c,j] h[c, t+j]  (t local)
        lg_ps = psum.tile([TLOC, M], F32)
        for j in range(MIXK):
            nc.tensor.matmul(
                lg_ps[:],
                h_sb[:, j : j + TLOC],
                mixw_sb[:, j * M : (j + 1) * M],
                start=(j == 0),
                stop=(j == MIXK - 1),
            )
        lt = small.tile([TLOC, M], F32)
        nc.vector.tensor_add(lt[:], lg_ps[:], mixb_sb[:])

        # softmax over m (free dim)
        rmax = small.tile([TLOC, 1], F32)
        nc.vector.tensor_reduce(rmax[:], lt[:], axis=mybir.AxisListType.X, op=AluOp.max)
        nmax = small.tile([TLOC, 1], F32)
        nc.vector.tensor_scalar(nmax[:], rmax[:], -1.0, None, AluOp.mult)
        ex = small.tile([TLOC, M], F32)
        nc.scalar.activation(ex[:], lt[:], ActFn.Exp, bias=nmax[:, 0:1], scale=1.0)
        ssum = small.tile([TLOC, 1], F32)
        nc.vector.tensor_reduce(ssum[:], ex[:], axis=mybir.AxisListType.X, op=AluOp.add)
        rcp = small.tile([TLOC, 1], F32)
        nc.vector.reciprocal(rcp[:], ssum[:])
        alpha_t = small.tile([TLOC, M], F32)
        nc.vector.tensor_scalar(alpha_t[:], ex[:], rcp[:, 0:1], None, AluOp.mult)

        # broadcast alpha to all 128 partitions: SBUF -> DRAM -> [1,48], then
        # PE outer product ones[128] x alpha[48] (K=1 matmul) -> PSUM -> SBUF
        nc.sync.dma_start(adram[:], alpha_t[:])
        a1 = small.tile([1, TLOC * M], F32)
        nc.sync.dma_start(a1[:], adram[:])
        ones = small.tile([1, C], F32)
        nc.vector.memset(ones[:], 1.0)
        abc_ps = psum.tile([C, TLOC * M], F32)
        nc.tensor.matmul(abc_ps[:], ones[:], a1[:], start=True, stop=True)
        abc = small.tile([C, TLOC * M], F32)
        nc.vector.tensor_copy(abc[:], abc_ps[:])

        # ---- fold alpha into per-frame effective filters ----
        # weff[c, t*54 + tap] = sum_m alpha[t, m] * wkv[c, m*54 + tap]
        weff = big.tile([C, TLOC * WBLK], F32)
        for t in range(TLOC):
            dst = weff[:, t * WBLK : (t + 1) * WBLK]
            for m in range(M):
                a_sc = abc[:, t * M + m : t * M + m + 1]
                src = wkv_sb[:, m * WBLK : (m + 1) * WBLK]
                if m == 0:
                    nc.vector.tensor_scalar(dst, src, a_sc, None, AluOp.mult)
                else:
                    nc.vector.scalar_tensor_tensor(
                        dst, src, a_sc, dst, AluOp.mult, AluOp.add
                    )

        # ---- the depthwise conv, 27 MACs per output frame ----
        k3 = k_sb[:].rearrange("p (t h w) -> p t h w", t=TIN, h=H)
        v3 = v_sb[:].rearrange("p (t h w) -> p t h w", t=TIN, h=H)

        for t in range(TLOC):
            for x3, base, odram in ((k3, 0, kout), (v3, NTAP, vout)):
                of = outp.tile([C, HW], F32, tag="of")
                o3 = of[:].rearrange("p (h w) -> p h w", h=H)
                wcol = lambda tap: weff[:, t * WBLK + base + tap : t * WBLK + base + tap + 1]
                # init with the full-coverage center tap (dt=0, dh=1, dw=1)
                nc.vector.tensor_scalar(o3[:, :, :], x3[:, t, :, :], wcol(4), None, AluOp.mult)
                for dt in range(KT):
                    for dh in range(KS):
                        for dw in range(KS):
                            if (dt, dh, dw) == (0, 1, 1):
                                continue
                            tap = dt * 9 + dh * 3 + dw
                            oh, ow = max(0, 1 - dh), max(0, 1 - dw)
                            ih, iw = max(0, dh - 1), max(0, dw - 1)
                            hc, wc = H - abs(dh - 1), W - abs(dw - 1)
                            nc.vector.scalar_tensor_tensor(
                                o3[:, oh : oh + hc, ow : ow + wc],
                                x3[:, t + dt, ih : ih + hc, iw : iw + wc],
                                wcol(tap),
                                o3[:, oh : oh + hc, ow : ow + wc],
                                AluOp.mult,
                                AluOp.add,
                            )
                nc.sync.dma_start(odram[:, t * HW : (t + 1) * HW], of[:])

    _split_sync_waits(nc)
    return nc


_PROGRAM_CACHE: bass.Bass | None = None

# Results of the last hardware run (for the test harness to inspect).
LAST_RESULT = None


def _get_program() -> bass.Bass:
    global _PROGRAM_CACHE
    if _PROGRAM_CACHE is None:
        _PROGRAM_CACHE = _build_program()
    return _PROGRAM_CACHE


def _halo_slice(x_b: np.ndarray, t0: int) -> np.ndarray:
    """x_b: [C, T, H, W] -> [C, TIN*HW] with 2 leading halo frames
    (zeros when t0 == 0)."""
    if t0 == 0:
        halo = np.zeros((C, THALO, H, W), dtype=x_b.dtype)
    else:
        halo = x_b[:, t0 - THALO : t0]
    out = np.concatenate([halo, x_b[:, t0 : t0 + TLOC]], axis=1)
    return np.ascontiguousarray(out.reshape(C, TIN * HW))


def _make_in_maps(q, k, v, Wk, Wv, pre_w, pre_b, mix_w, mix_b):
    q = np.asarray(q, dtype=np.float32)
    k = np.asarray(k, dtype=np.float32)
    v = np.asarray(v, dtype=np.float32)
    Wk = np.asarray(Wk, dtype=np.float32)
    Wv = np.asarray(Wv, dtype=np.float32)
    pre_w = np.asarray(pre_w, dtype=np.float32)
    pre_b = np.asarray(pre_b, dtype=np.float32)
    mix_w = np.asarray(mix_w, dtype=np.float32)
    mix_b = np.asarray(mix_b, dtype=np.float32)

    # shared (replicated) weight prep
    wk_flat = Wk.reshape(M, C, NTAP)  # [m, c, tap]
    wv_flat = Wv.reshape(M, C, NTAP)
    wkv_host = np.empty((C, M * WBLK), dtype=np.float32)
    for m in range(M):
        wkv_host[:, m * WBLK : m * WBLK + NTAP] = wk_flat[m].reshape(C, NTAP)
        wkv_host[:, m * WBLK + NTAP : (m + 1) * WBLK] = wv_flat[m].reshape(C, NTAP)
    prew_host = np.ascontiguousarray((pre_w / HW).T)  # [c_in, c_out]
    preb_host = np.ascontiguousarray(pre_b.reshape(C, 1))
    mixw_host = np.empty((C, MIXK * M), dtype=np.float32)
    for j in range(MIXK):
        for m in range(M):
            mixw_host[:, j * M + m] = mix_w[m, :, j]
    mixb_host = np.ascontiguousarray(np.tile(mix_b[None, :], (TLOC, 1)))

    in_maps = []
    for core in range(NCORES):
        b, th = core // 2, core % 2
        t0 = th * TLOC
        hm = np.zeros((C, THALO), np.float32) if t0 == 0 else np.ones((C, THALO), np.float32)
        in_maps.append(
            {
                "qin": _halo_slice(q[b], t0),
                "kin": _halo_slice(k[b], t0),
                "vin": _halo_slice(v[b], t0),
                "wkv": wkv_host,
                "prew": prew_host,
                "preb": preb_host,
                "mixw": mixw_host,
                "mixb": mixb_host,
                "hmask": hm,
            }
        )
    return in_maps


def kernel(q, k, v, Wk, Wv, pre_w, pre_b, mix_w, mix_b):
    in_maps = _make_in_maps(q, k, v, Wk, Wv, pre_w, pre_b, mix_w, mix_b)
    nc = _get_program()
    trace = bool(int(os.environ.get("BASSK_TRACE", "0")))
    res = run_bass_kernel_spmd(nc, in_maps, list(range(NCORES)), trace=trace)
    global LAST_RESULT
    LAST_RESULT = res

    k_out = np.empty((B, C, T, H, W), dtype=np.float32)
    v_out = np.empty((B, C, T, H, W), dtype=np.float32)
    for core in range(NCORES):
        b, th = core // 2, core % 2
        t0 = th * TLOC
        k_out[b, :, t0 : t0 + TLOC] = res.results[core]["kout"].reshape(C, TLOC, H, W)
        v_out[b, :, t0 : t0 + TLOC] = res.results[core]["vout"].reshape(C, TLOC, H, W)
    return (k_out, v_out)



# revision 15
# speedup vs baseline: 2.9645x; 2.9645x over previous
"""Trainium2 Bass kernel for nn_ConvKernelBank.

Computation:
  alpha = softmax_M( causal_conv1d( gelu(pre_w @ mean_hw(q) + pre_b), mix_w ) + mix_b )
  k_out = sum_m alpha[b,m,t] * dwconv3d_causal(k, Wk[m])
  v_out = sum_m alpha[b,m,t] * dwconv3d_causal(v, Wv[m])

Strategy:
  - 8 NeuronCores, data-parallel over (batch, T-half): core i handles
    b = i // 2, t in [16*(i%2), 16*(i%2)+16).  Causal temporal halo of 2
    frames is passed in from the host (zeros at sequence start).
  - The mixture weights alpha[m, t] are folded into per-output-frame
    effective 27-tap depthwise filters (3x less work than 3 branches).
  - k/v arrive fp16 and spatially ZERO-PADDED to 26x26 on the host, so
    every tap reads a full unclipped 24x24 window.
  - The 864 tap-MACs per core (2 tensors x 16 frames x 27 taps, each a
    [128, 576] per-channel multiply-accumulate) are split across ALL
    four compute engines:
      * PE: depthwise scaling as diagonal-matmul: psum += diag(w) @ x_win.
        fp16 moving data -> 1 PE cycle per output row (240 ns per tap).
        Diagonal matrices are built on the fly by DVE (tensor_scalar in
        4x mode, 93 ns) or ACT (activation Copy with per-partition
        scale, 292 ns) from an identity matrix.
      * DVE: scalar_tensor_tensor mult-add into an fp32 SBUF accumulator.
      * Pool (GpSimd): same MACs via its scalar_tensor_tensor.
      * ACT: drains PE's PSUM accumulators into the fp16 staging buffer.
  - fp16 input + output DMA (halved traffic); fp32 accumulation
    everywhere (PSUM / SBUF accumulators).
"""

import os
from contextlib import ExitStack

import numpy as np

import concourse.bass as bass
import concourse.tile as tile
from concourse import mybir
from concourse.bass_utils import run_bass_kernel_spmd
from concourse.vector_clock import ScopedClock

B, C, T, H, W = 4, 128, 32, 24, 24
M, KT, KS, MIXK = 3, 3, 3, 3
HW = H * W
NCORES = 8
TLOC = 16          # output frames per core
THALO = 2          # causal temporal halo
TIN = TLOC + THALO
F32 = mybir.dt.float32
F16 = mybir.dt.float16
NTAP = KT * KS * KS  # 27
# per-frame effective filter block: 27 k-taps then 27 v-taps
WBLK = 2 * NTAP      # 54
HP, WP = H + 2, W + 2        # zero-padded spatial dims
HWP = HP * WP                # 676

AluOp = mybir.AluOpType
ActFn = mybir.ActivationFunctionType

# ---- engine assignment ----------------------------------------------------
# 32 units = (frame t, tensor in {k,v}).  unit index u = 2*t + xi.
# Remaining units go to PE.  Tuned against the TimelineSim cost model.
# (walrus rejects TensorScalarPtr on the Pool engine, so no Pool MACs.)
_POOL_UNITS = frozenset()
_DVE_UNITS = frozenset({3, 8, 13, 18, 23, 27, 31})


def _diag_on_dve(pe_idx: int) -> bool:
    """Which PE units' diagonals DVE builds (rest built by ACT): every 4th
    early on, leaning harder on DVE near the tail where its STT units end."""
    if pe_idx >= 20:
        return pe_idx % 2 == 0
    return pe_idx % 4 == 3


class _SplitDrainTileContext(tile.TileContext):
    """TileContext whose final drain splits semaphore waits across several
    drain instructions: this walrus build rejects >2 sync waits on one
    CTRL instruction ("Too many sync wait commands")."""

    MAX_WAITS = 1

    def _drain_and_barrier(self, tick_clock, wait_clock):
        nc = self.nc
        drain_inst = nc.sync.drain()
        wait_clock.add_sem_waits(
            drain_inst.ins, ScopedClock({None: tick_clock.global_clock})
        )
        mi = drain_inst.ins
        si = mi.sync_info
        waits = list(si.on_wait or []) if si is not None else []
        if len(waits) > self.MAX_WAITS:
            si.on_wait = waits[: self.MAX_WAITS]
            rest = waits[self.MAX_WAITS :]
            for i in range(0, len(rest), self.MAX_WAITS):
                d2 = nc.sync.drain()
                d2.ins.sync_info = mybir.SyncInfo(
                    on_wait=rest[i : i + self.MAX_WAITS], on_update=[]
                )
        nc.all_engine_barrier()
        popped = nc._tile_sem_poison_stack.pop()
        assert popped is self._sem_poison
        nc.clear_and_free_semaphores(list(self.sems.allocated().values()))
        nc.all_engine_barrier()


_MAX_SYNC_WAITS = 1

_NOP_ENGINES = {
    mybir.EngineType.PE,
    mybir.EngineType.DVE,
    mybir.EngineType.Activation,
    mybir.EngineType.Pool,
    mybir.EngineType.SP,
}


def _split_sync_waits(nc: bass.Bass, max_waits: int = _MAX_SYNC_WAITS) -> None:
    """Walrus rejects instructions carrying more than ~2 semaphore waits.
    Move excess waits onto freshly inserted same-engine NoOps placed just
    before the offending instruction (waiting earlier is always safe)."""
    for fn in nc.m.functions:
        for bb in fn.blocks:
            new_list = []
            changed = False
            for inst in bb.instructions:
                si = inst.sync_info
                waits = list(si.on_wait) if (si is not None and si.on_wait) else []
                if len(waits) > max_waits:
                    assert inst.engine in _NOP_ENGINES, (
                        f"can't split waits on {inst.engine} {type(inst).__name__}"
                    )
                    excess, keep = waits[:-max_waits], waits[-max_waits:]
                    for i in range(0, len(excess), max_waits):
                        nop = mybir.InstNoOp(
                            name=nc.get_next_instruction_name(), ins=[], outs=[]
                        )
                        nop.engine = inst.engine
                        nop.sync_info = mybir.SyncInfo(
                            on_wait=excess[i : i + max_waits], on_update=[]
                        )
                        new_list.append(nop)
                    si.on_wait = keep
                    changed = True
                new_list.append(inst)
            if changed:
                bb.instructions[:] = new_list


def _build_program() -> bass.Bass:
    nc = bass.Bass()

    qin = nc.declare_dram_parameter("qin", [C, TIN * HW], F16, isOutput=False)
    kin = nc.declare_dram_parameter("kin", [C, TIN * HWP], F16, isOutput=False)
    vin = nc.declare_dram_parameter("vin", [C, TIN * HWP], F16, isOutput=False)
    # single fp32 constant blob [C, 302]:
    #   [0:162)   wkv   [c, m*54 + tap] (tap 0..26 = Wk[m,c], 27..53 = Wv[m,c])
    #   [162:290) prew  (pre_w / HW).T  -> lhsT layout [c_in, c_out]
    #   [290:291) preb
    #   [291:300) mixw  [c, j*3 + m] = mix_w[m, c, j]
    #   [300:302) hmask (0 for the 2 halo cols on sequence-start cores else 1)
    CBLOB = M * WBLK + C + 1 + MIXK * M + THALO
    cblob = nc.declare_dram_parameter("cblob", [C, CBLOB], F32, isOutput=False)
    # mix_b tiled to [TLOC, M]
    mixb = nc.declare_dram_parameter("mixb", [TLOC, M], F32, isOutput=False)
    ident = nc.declare_dram_parameter("ident", [C, C], F16, isOutput=False)

    kout = nc.declare_dram_parameter("kout", [C, TLOC * HW], F16, isOutput=True)
    vout = nc.declare_dram_parameter("vout", [C, TLOC * HW], F16, isOutput=True)

    with ExitStack() as ctx:
        tc = ctx.enter_context(_SplitDrainTileContext(nc))

        consts = ctx.enter_context(tc.tile_pool(name="consts", bufs=1))
        big = ctx.enter_context(tc.tile_pool(name="big", bufs=1))
        small = ctx.enter_context(tc.tile_pool(name="small", bufs=1))
        diagp_a = ctx.enter_context(tc.tile_pool(name="diagp_a", bufs=10))
        diagp_d = ctx.enter_context(tc.tile_pool(name="diagp_d", bufs=8))
        accp = ctx.enter_context(tc.tile_pool(name="accp", bufs=4))
        psum_pro = ctx.enter_context(tc.tile_pool(name="psum_pro", bufs=1, space="PSUM"))
        psum_cv = ctx.enter_context(tc.tile_pool(name="psum_cv", bufs=7, space="PSUM"))

        # ---- load bulk data (q first: the mix predictor gates everything;
        # chunked so per-frame pooling overlaps the transfer) ----
        q_sb = big.tile([C, TIN * HW], F16)
        QCH = 3  # q DMA chunks
        qch = TIN // QCH
        for i in range(QCH):
            s = i * qch * HW
            e = (i + 1) * qch * HW if i < QCH - 1 else TIN * HW
            nc.sync.dma_start(q_sb[:, s:e], qin[:, s:e])
        cblob_sb = consts.tile([C, CBLOB], F32)
        nc.sync.dma_start(cblob_sb[:], cblob[:])
        mixb_sb = consts.tile([TLOC, M], F32)
        nc.sync.dma_start(mixb_sb[:], mixb[:])
        ident_sb = consts.tile([C, C], F16)
        nc.sync.dma_start(ident_sb[:], ident[:])

        o = 0
        wkv_sb = cblob_sb[:, o : o + M * WBLK]; o += M * WBLK
        prew_sb = cblob_sb[:, o : o + C]; o += C
        preb_sb = cblob_sb[:, o : o + 1]; o += 1
        mixw_sb = cblob_sb[:, o : o + MIXK * M]; o += MIXK * M
        hmask_sb = cblob_sb[:, o : o + THALO]; o += THALO

        # preload the Gelu activation table while q streams in
        ones = small.tile([1, C], F32)
        nc.vector.memset(ones[:], 1.0)
        gdum = small.tile([1, C], F32)
        nc.scalar.activation(gdum[:], ones[:], ActFn.Gelu)

        # Early k/v chunks (frames 0-8) only; the rest are issued AFTER the
        # alpha gather DMA below, so the gather isn't queued behind them on
        # the DMA engines (SP blocks on the gather's sem wait, which is fine:
        # late frames aren't needed until much later).
        k_sb = big.tile([C, TIN * HWP], F16)
        v_sb = big.tile([C, TIN * HWP], F16)
        _in_chunks = [(0, 5 * HWP), (5 * HWP, 9 * HWP),
                      (9 * HWP, 13 * HWP), (13 * HWP, TIN * HWP)]
        for s, e in _in_chunks[:2]:
            nc.sync.dma_start(k_sb[:, s:e], kin[:, s:e])
            nc.sync.dma_start(v_sb[:, s:e], vin[:, s:e])

        # ---- mix predictor ----
        # qg[c, t] = sum_hw q  (the 1/HW is folded into prew on host).
        # Split the per-frame reductions between DVE and ACT, per q chunk.
        qg = small.tile([C, TIN], F32)
        qtrash = small.tile([C, HW], F16)
        for t in range(TIN):
            src = q_sb[:, t * HW : (t + 1) * HW]
            if t % 2 == 0:
                nc.vector.tensor_reduce(
                    qg[:, t : t + 1], src, axis=mybir.AxisListType.X, op=AluOp.add
                )
            else:
                nc.scalar.activation(
                    qtrash[:], src, ActFn.Copy, accum_out=qg[:, t : t + 1]
                )
        # h = gelu(prew.T @ qg + preb)
        h_ps = psum_pro.tile([C, TLOC * M], F32, tag="pro", name="h_ps_t")[:, 0:TIN]
        nc.tensor.matmul(h_ps[:], prew_sb[:], qg[:], start=True, stop=True)
        h_sb = small.tile([C, TIN], F32)
        nc.scalar.activation(
            h_sb[:], h_ps[:], ActFn.Gelu, bias=preb_sb[:, 0:1], scale=1.0
        )
        # zero the causal halo columns where the reference zero-pads h
        nc.vector.tensor_mul(h_sb[:, 0:THALO], h_sb[:, 0:THALO], hmask_sb[:])

        # logits[t, m] = sum_j sum_c mix_w[m,c,j] h[c, t+j]  (t local)
        lg_ps = psum_pro.tile([C, TLOC * M], F32, tag="pro", name="lg_ps_t")[0:TLOC, 0:M]
        for j in range(MIXK):
            nc.tensor.matmul(
                lg_ps[:],
                h_sb[:, j : j + TLOC],
                mixw_sb[:, j * M : (j + 1) * M],
                start=(j == 0),
                stop=(j == MIXK - 1),
            )
        lt = small.tile([TLOC, M], F32)
        nc.vector.tensor_add(lt[:], lg_ps[:], mixb_sb[:])

        # softmax over m (free dim)
        rmax = small.tile([TLOC, 1], F32)
        nc.vector.tensor_reduce(rmax[:], lt[:], axis=mybir.AxisListType.X, op=AluOp.max)
        nmax = small.tile([TLOC, 1], F32)
        nc.vector.tensor_scalar(nmax[:], rmax[:], -1.0, None, AluOp.mult)
        ex = small.tile([TLOC, M], F32)
        nc.scalar.activation(ex[:], lt[:], ActFn.Exp, bias=nmax[:, 0:1], scale=1.0)
        ssum = small.tile([TLOC, 1], F32)
        nc.vector.tensor_reduce(ssum[:], ex[:], axis=mybir.AxisListType.X, op=AluOp.add)
        rcp = small.tile([TLOC, 1], F32)
        nc.vector.reciprocal(rcp[:], ssum[:])
        alpha_t = small.tile([TLOC, M], F32)
        nc.vector.tensor_scalar(alpha_t[:], ex[:], rcp[:, 0:1], None, AluOp.mult)

        # broadcast alpha to all 128 partitions: SBUF->SBUF DMA gathers the
        # [16, 3] tile into one [1, 48] row, then PE outer product
        # ones[128] x alpha[48] (K=1 matmul) -> PSUM -> SBUF
        a1 = small.tile([1, TLOC * M], F32)
        nc.sync.dma_start(a1[:], alpha_t[:])
        for s, e in _in_chunks[2:]:
            nc.sync.dma_start(k_sb[:, s:e], kin[:, s:e])
            nc.sync.dma_start(v_sb[:, s:e], vin[:, s:e])
        abc_ps = psum_pro.tile([C, TLOC * M], F32, tag="pro", name="abc_ps_t")
        nc.tensor.matmul(abc_ps[:], ones[:], a1[:], start=True, stop=True)
        abc = small.tile([C, TLOC * M], F32)
        nc.vector.tensor_copy(abc[:], abc_ps[:])

        # ---- fold alpha into per-frame effective filters ----
        # weff[c, t*54 + tap] = sum_m alpha[t, m] * wkv[c, m*54 + tap]
        weff = big.tile([C, TLOC * WBLK], F32)
        for t in range(TLOC):
            dst = weff[:, t * WBLK : (t + 1) * WBLK]
            for m in range(M):
                a_sc = abc[:, t * M + m : t * M + m + 1]
                src = wkv_sb[:, m * WBLK : (m + 1) * WBLK]
                if m == 0:
                    nc.vector.tensor_scalar(dst, src, a_sc, None, AluOp.mult)
                else:
                    nc.vector.scalar_tensor_tensor(
                        dst, src, a_sc, dst, AluOp.mult, AluOp.add
                    )

        # ---- the depthwise conv ----
        k4 = k_sb[:].rearrange("p (t h w) -> p t h w", t=TIN, h=HP)
        v4 = v_sb[:].rearrange("p (t h w) -> p t h w", t=TIN, h=HP)
        kstage = big.tile([C, TLOC * HW], F16)
        vstage = big.tile([C, TLOC * HW], F16)

        HH = H // 2  # rows per matmul half (moving free dim 288 <= 512)
        n_pe_seen = 0
        done_units = []  # (t, xi) in completion-emission order

        for t in range(TLOC):
            for xi, (x4, stage, base) in enumerate(
                ((k4, kstage, 0), (v4, vstage, NTAP))
            ):
                u = 2 * t + xi
                wcol = lambda tap: weff[
                    :, t * WBLK + base + tap : t * WBLK + base + tap + 1
                ]

                if u in _DVE_UNITS:
                    acc = accp.tile([C, HW], F32, tag="acc")
                    a3 = acc[:].rearrange("p (h w) -> p h w", h=H)
                    for tap in range(NTAP):
                        dt, dh, dw = tap // 9, (tap % 9) // 3, tap % 3
                        win = x4[:, t + dt, dh : dh + H, dw : dw + W]
                        if tap == 0:
                            nc.vector.tensor_scalar(
                                a3[:, :, :], win, wcol(0), None, AluOp.mult
                            )
                        else:
                            nc.vector.scalar_tensor_tensor(
                                a3[:, :, :], win, wcol(tap), a3[:, :, :],
                                AluOp.mult, AluOp.add,
                            )
                    dst = stage[:, t * HW : (t + 1) * HW]
                    nc.vector.tensor_copy(dst, acc[:])
                else:
                    # PE unit: 27 diagonal matmuls accumulate in PSUM
                    ph0 = psum_cv.tile([C, HH * W], F32, tag="ph")
                    ph1 = psum_cv.tile([C, HH * W], F32, tag="ph")
                    diag_on_dve = _diag_on_dve(n_pe_seen)
                    n_pe_seen += 1
                    for tap in range(NTAP):
                        dt, dh, dw = tap // 9, (tap % 9) // 3, tap % 3
                        pool_ = diagp_d if diag_on_dve else diagp_a
                        diag = pool_.tile([C, C], F16, tag="diag")
                        if diag_on_dve:
                            nc.vector.tensor_scalar(
                                diag[:], ident_sb[:], wcol(tap), None, AluOp.mult
                            )
                        else:
                            nc.scalar.activation(
                                diag[:], ident_sb[:], ActFn.Copy, scale=wcol(tap)
                            )
                        st, sp = tap == 0, tap == NTAP - 1
                        nc.tensor.matmul(
                            ph0[:], diag[:],
                            x4[:, t + dt, dh : dh + HH, dw : dw + W],
                            start=st, stop=sp,
                        )
                        nc.tensor.matmul(
                            ph1[:], diag[:],
                            x4[:, t + dt, dh + HH : dh + H, dw : dw + W],
                            start=st, stop=sp,
                        )
                    o = t * HW
                    nc.scalar.activation(
                        stage[:, o : o + HH * W], ph0[:], ActFn.Copy
                    )
                    nc.scalar.activation(
                        stage[:, o + HH * W : o + HW], ph1[:], ActFn.Copy
                    )
                done_units.append((t, xi))

            # output DMA in 4-frame chunks
            if t % 4 == 3:
                c0, c1 = (t - 3) * HW, (t + 1) * HW
                nc.sync.dma_start(kout[:, c0:c1], kstage[:, c0:c1])
                nc.sync.dma_start(vout[:, c0:c1], vstage[:, c0:c1])

    _split_sync_waits(nc)
    return nc


_PROGRAM_CACHE: bass.Bass | None = None

# Results of the last hardware run (for the test harness to inspect).
LAST_RESULT = None


def _get_program() -> bass.Bass:
    global _PROGRAM_CACHE
    if _PROGRAM_CACHE is None:
        _PROGRAM_CACHE = _build_program()
    return _PROGRAM_CACHE


def _halo_slice(x_b: np.ndarray, t0: int) -> np.ndarray:
    """x_b: [C, T, H, W] -> [C, TIN*HW] fp16 with 2 leading halo frames
    (zeros when t0 == 0)."""
    if t0 == 0:
        halo = np.zeros((C, THALO, H, W), dtype=np.float16)
    else:
        halo = x_b[:, t0 - THALO : t0].astype(np.float16)
    out = np.concatenate([halo, x_b[:, t0 : t0 + TLOC].astype(np.float16)], axis=1)
    return np.ascontiguousarray(out.reshape(C, TIN * HW))


def _halo_pad_slice(x_b: np.ndarray, t0: int) -> np.ndarray:
    """x_b: [C, T, H, W] -> [C, TIN*HWP] fp16: 2 leading halo frames and
    each frame zero-padded to 26x26."""
    out = np.zeros((C, TIN, HP, WP), dtype=np.float16)
    lo = max(t0 - THALO, 0)
    out[:, THALO - (t0 - lo) :, 1 : 1 + H, 1 : 1 + W] = x_b[
        :, lo : t0 + TLOC
    ].astype(np.float16)
    return np.ascontiguousarray(out.reshape(C, TIN * HWP))


def _make_in_maps(q, k, v, Wk, Wv, pre_w, pre_b, mix_w, mix_b):
    q = np.asarray(q, dtype=np.float32)
    k = np.asarray(k, dtype=np.float32)
    v = np.asarray(v, dtype=np.float32)
    Wk = np.asarray(Wk, dtype=np.float32)
    Wv = np.asarray(Wv, dtype=np.float32)
    pre_w = np.asarray(pre_w, dtype=np.float32)
    pre_b = np.asarray(pre_b, dtype=np.float32)
    mix_w = np.asarray(mix_w, dtype=np.float32)
    mix_b = np.asarray(mix_b, dtype=np.float32)

    # shared (replicated) weight prep
    wk_flat = Wk.reshape(M, C, NTAP)  # [m, c, tap]
    wv_flat = Wv.reshape(M, C, NTAP)
    wkv_host = np.empty((C, M * WBLK), dtype=np.float32)
    for m in range(M):
        wkv_host[:, m * WBLK : m * WBLK + NTAP] = wk_flat[m].reshape(C, NTAP)
        wkv_host[:, m * WBLK + NTAP : (m + 1) * WBLK] = wv_flat[m].reshape(C, NTAP)
    prew_host = np.ascontiguousarray((pre_w / HW).T)  # [c_in, c_out]
    preb_host = np.ascontiguousarray(pre_b.reshape(C, 1))
    mixw_host = np.empty((C, MIXK * M), dtype=np.float32)
    for j in range(MIXK):
        for m in range(M):
            mixw_host[:, j * M + m] = mix_w[m, :, j]
    mixb_host = np.ascontiguousarray(np.tile(mix_b[None, :], (TLOC, 1)))
    ident_host = np.eye(C, dtype=np.float16)

    in_maps = []
    for core in range(NCORES):
        b, th = core // 2, core % 2
        t0 = th * TLOC
        hm = np.zeros((C, THALO), np.float32) if t0 == 0 else np.ones((C, THALO), np.float32)
        cblob_host = np.ascontiguousarray(
            np.concatenate([wkv_host, prew_host, preb_host, mixw_host, hm], axis=1)
        )
        in_maps.append(
            {
                "qin": _halo_slice(q[b], t0),
                "kin": _halo_pad_slice(k[b], t0),
                "vin": _halo_pad_slice(v[b], t0),
                "cblob": cblob_host,
                "mixb": mixb_host,
                "ident": ident_host,
            }
        )
    return in_maps


def kernel(q, k, v, Wk, Wv, pre_w, pre_b, mix_w, mix_b):
    in_maps = _make_in_maps(q, k, v, Wk, Wv, pre_w, pre_b, mix_w, mix_b)
    nc = _get_program()
    trace = bool(int(os.environ.get("BASSK_TRACE", "0")))
    res = run_bass_kernel_spmd(nc, in_maps, list(range(NCORES)), trace=trace)
    global LAST_RESULT
    LAST_RESULT = res

    k_out = np.empty((B, C, T, H, W), dtype=np.float32)
    v_out = np.empty((B, C, T, H, W), dtype=np.float32)
    for core in range(NCORES):
        b, th = core // 2, core % 2
        t0 = th * TLOC
        k_out[b, :, t0 : t0 + TLOC] = (
            res.results[core]["kout"].astype(np.float32).reshape(C, TLOC, H, W)
        )
        v_out[b, :, t0 : t0 + TLOC] = (
            res.results[core]["vout"].astype(np.float32).reshape(C, TLOC, H, W)
        )
    return (k_out, v_out)
